# revision 18
# baseline (speedup 1.0000x reference)
"""Sliding-window MHA Trainium2 kernel, sharded over 8 NeuronCores.

Problem (hardcoded): B=2, L=2048, D=1024, H=16 heads (hd=64), window
|i-j| <= 256, fp32 I/O.

Sharding: core = b*4 + g  (b in 0..1 batches, g in 0..3 head-groups of 4
heads). Each core computes QKV projection for its 4 heads, banded
attention, and a partial output projection (its 256 columns of the head
concat). Host sums the 4 partial projections per batch and adds out_b.

Device pipeline per core (matmuls fp32r, E/V fp16):
  x^T (host-pretransposed) -> Q^T,K^T [f,t] and V [t,f] projections
  per head: per k-block S^T = K^T.T Q^T over the 640-wide band window,
  E^T = exp(S/8) fp16 with static triangle masks on edge strips,
  O [q,65] = E^T-chunks.T @ [V|1] accumulated in PSUM, normalized by
  col 64, PE-transposed to O'^T, projected with the out_w slice.
  All phases are software-pipelined in emission order so PE (matmul),
  ACT (exp), DVE (copies) and DMA overlap.
"""

import numpy as np

import concourse.bacc as bacc
import concourse.mybir as mybir
import concourse.tile as tile
from concourse.bass_utils import run_bass_kernel_spmd
from concourse.masks import make_identity

F32 = mybir.dt.float32
F32R = mybir.dt.float32r
F16 = mybir.dt.float16

P = 128
L = 2048
D = 1024
NH = 4          # heads per core
HD = 64
FQK = 512       # q+k feature rows per core (4 heads * 64 * 2)
FV = 256        # v feature rows per core
WIN = 256
KB = L // P     # 16 k-blocks
N_CORES = 8


def _window(kb):
    """q-range [qlo, qhi) covered by k-block kb under |q-k| <= WIN."""
    k0 = kb * P
    qlo = max(0, k0 - WIN)
    qhi = min(L, k0 + P + WIN)
    return qlo, qhi - qlo


def _build_nc():
    nc = bacc.Bacc(
        "TRN2", target_bir_lowering=False, debug=False, num_devices=N_CORES
    )
    xt_d = nc.dram_tensor("xt", [D, L], F16, kind="ExternalInput").ap()
    wqk_d = nc.dram_tensor("wqk_t", [4, D, P], F16, kind="ExternalInput").ap()
    wv_d = nc.dram_tensor("wv_t", [D, FV], F16, kind="ExternalInput").ap()
    wo_d = nc.dram_tensor("wo_t", [FV, D], F32R, kind="ExternalInput").ap()
    bqk_d = nc.dram_tensor("bqk", [P, 4], F32, kind="ExternalInput").ap()
    bv_d = nc.dram_tensor("bv", [1, FV], F32, kind="ExternalInput").ap()
    y_d = nc.dram_tensor("y", [L, D], F32, kind="ExternalOutput").ap()

    with tile.TileContext(nc) as tc:
        _emit(nc, tc, xt_d, wqk_d, wv_d, wo_d, bqk_d, bv_d, y_d)
    nc.compile()
    return nc


def _emit(nc, tc, xt_d, wqk_d, wv_d, wo_d, bqk_d, bv_d, y_d):
    import contextlib

    ctx = contextlib.ExitStack()
    with ctx:
        const = ctx.enter_context(tc.tile_pool(name="const", bufs=1))
        w_pool = ctx.enter_context(tc.tile_pool(name="w", bufs=1))
        qk_pool = ctx.enter_context(tc.tile_pool(name="qk", bufs=1))
        v_pool = ctx.enter_context(tc.tile_pool(name="v", bufs=1))
        xt_pool = ctx.enter_context(tc.tile_pool(name="xt", bufs=1))
        e_pool = ctx.enter_context(tc.tile_pool(name="e", bufs=28))
        oT_pool = ctx.enter_context(tc.tile_pool(name="oT", bufs=1))
        sm_pool = ctx.enter_context(tc.tile_pool(name="sm", bufs=12))
        ysb_pool = ctx.enter_context(tc.tile_pool(name="ysb", bufs=3))
        ppool = ctx.enter_context(tc.tile_pool(name="ppsum", bufs=2, space="PSUM"))

        # ---- weight/bias/x DMAs (fine-grained, pipeline-ordered) --------
        wqk_sb = w_pool.tile([P, 4, 8, P], F16)
        wv_sb = w_pool.tile([P, 8, FV], F16)
        wo_sb = w_pool.tile([P, 2, D], F32R)
        xt_sb = xt_pool.tile([P, 8, L], F16)
        wqk_re = wqk_d.rearrange("f (c p) n -> p f c n", p=P)
        xt_re = xt_d.rearrange("(c p) t -> p c t", p=P)

        bqk_sb = const.tile([P, 4], F32)
        nc.sync.dma_start(bqk_sb[:], bqk_d[:])
        bv_row = const.tile([1, FV], F32)
        nc.sync.dma_start(bv_row[:], bv_d[:])

        # first compute wave needs wqk fc0/fc2 + xt t-slice 0
        nc.sync.dma_start(wqk_sb[:, 0, :, :], wqk_re[:, 0, :, :])
        for dc in range(8):
            nc.sync.dma_start(xt_sb[:, dc, 0:512], xt_re[:, dc, 0:512])
        nc.sync.dma_start(wqk_sb[:, 2, :, :], wqk_re[:, 2, :, :])
        nc.sync.dma_start(wv_sb[:], wv_d.rearrange("(c p) n -> p c n", p=P))
        for t in range(1, 4):
            for dc in range(8):
                nc.sync.dma_start(
                    xt_sb[:, dc, t * 512:(t + 1) * 512],
                    xt_re[:, dc, t * 512:(t + 1) * 512])
        for fc in (1, 3):
            nc.sync.dma_start(wqk_sb[:, fc, :, :], wqk_re[:, fc, :, :])
        nc.sync.dma_start(wo_sb[:], wo_d.rearrange("(c p) n -> p c n", p=P))

        # ---- constants --------------------------------------------------
        ident_f32 = const.tile([P, P], F32)
        make_identity(nc, ident_f32[:])
        ident = const.tile([P, P], F32R)
        nc.vector.tensor_copy(ident[:], ident_f32[:])
        # mask_l[kl, c] = 1 if c >= kl else 0   (upper tri incl diag)
        mask_l = const.tile([P, P], F16)
        nc.gpsimd.memset(mask_l[:], 1.0)
        nc.gpsimd.affine_select(
            out=mask_l[:], in_=mask_l[:],
            compare_op=mybir.AluOpType.is_ge, fill=0.0,
            base=0, pattern=[[1, P]], channel_multiplier=-1,
        )
        # mask_r[kl, c] = 1 if c <= kl else 0   (lower tri incl diag)
        mask_r = const.tile([P, P], F16)
        nc.gpsimd.memset(mask_r[:], 1.0)
        nc.gpsimd.affine_select(
            out=mask_r[:], in_=mask_r[:],
            compare_op=mybir.AluOpType.is_ge, fill=0.0,
            base=0, pattern=[[-1, P]], channel_multiplier=1,
        )
        bv_bc = const.tile([P, FV], F32)
        nc.gpsimd.partition_broadcast(bv_bc[:], bv_row[:])

        # Q^T / K^T per head-pair chunk: [f%128, chunk, t]
        qT = qk_pool.tile([P, 2, L], F32R)
        kT = qk_pool.tile([P, 2, L], F32R)
        # V per k-block, heads side by side, each with ones col (rowsum)
        v_ext = v_pool.tile([P, KB, NH * (HD + 1)], F16)
        nc.gpsimd.memset(v_ext[:], 1.0)
        oT = oT_pool.tile([P, 2, L], F32R)

        # ---- emission helpers -------------------------------------------
        def qk_proj(fc, t):
            pq = ppool.tile([P, 512], F32, tag="pqk", name="pq")
            for dc in range(8):
                nc.tensor.matmul(
                    pq[:],
                    lhsT=wqk_sb[:, fc, dc, :],
                    rhs=xt_sb[:, dc, t * 512:(t + 1) * 512],
                    start=(dc == 0), stop=(dc == 7),
                )
            dest = qT if fc < 2 else kT
            nc.vector.tensor_scalar_add(
                dest[:, fc % 2, t * 512:(t + 1) * 512], pq[:],
                bqk_sb[:, fc:fc + 1],
            )

        def v_proj(t):
            pv = ppool.tile([P, 512], F32, tag="pqk", name="pv")
            for dc in range(8):
                nc.tensor.matmul(
                    pv[:, 0:FV],
                    lhsT=xt_sb[:, dc, t * P:(t + 1) * P],
                    rhs=wv_sb[:, dc, :],
                    start=(dc == 0), stop=(dc == 7),
                )
            nc.vector.tensor_add(
                v_ext[:, t, :].rearrange("p (h c) -> p h c", h=NH)[:, :, 0:HD],
                pv[:, 0:FV].rearrange("p (h c) -> p h c", h=NH),
                bv_bc[:].rearrange("p (h c) -> p h c", h=NH),
            )

        def phase_b(h, kb, e_tiles):
            cc, po = h // 2, (h % 2) * HD
            qlo, w = _window(kb)
            e_sb = e_pool.tile([P, 640], F16, tag="e", name="e_sb")
            e_tiles[(h, kb)] = e_sb
            s_ps = ppool.tile([P, 1024], F32, tag="s", name="s_ps")
            if w == 640:
                pieces = [(0, 0, 320), (320, 512, 320)]
            else:
                pieces = [(0, 0, w)]
            for qoff, poff, pw in pieces:
                nc.tensor.matmul(
                    s_ps[:, poff:poff + pw],
                    lhsT=kT[po:po + HD, cc, kb * P:(kb + 1) * P],
                    rhs=qT[po:po + HD, cc, qlo + qoff:qlo + qoff + pw],
                    start=True, stop=True,
                )
            if w == 640:
                src = s_ps[:].rearrange("p (g c) -> p g c", g=2)[:, :, 0:320]
                dst = e_sb[:].rearrange("p (g c) -> p g c", g=2)
            else:
                src = s_ps[:, 0:w]
                dst = e_sb[0:P, 0:w]
            nc.scalar.activation(
                dst, src, mybir.ActivationFunctionType.Exp, scale=0.125)
            if kb >= 2:
                nc.gpsimd.tensor_mul(e_sb[:, 0:P], e_sb[:, 0:P], mask_l[:])
            if kb <= KB - 3:
                nc.gpsimd.tensor_mul(
                    e_sb[:, w - P:w], e_sb[:, w - P:w], mask_r[:])

        def phase_c1(h, qt, e_tiles, ot_store, oq_store):
            cc, po = h // 2, (h % 2) * HD
            kbs = range(max(0, qt - 2), min(KB, qt + 3))
            if h == 0:
                ot = ppool.tile([P, 1024], F32, tag="ot", name="ot", bufs=1)
                ot_store[qt] = ot
            else:
                ot = ot_store[qt]
            o_ps = ot[:, h * 65:h * 65 + 65]
            for i, kb in enumerate(kbs):
                qlo, w = _window(kb)
                off = qt * P - qlo
                nc.tensor.matmul(
                    o_ps,
                    lhsT=e_tiles[(h, kb)][:, off:off + P],
                    rhs=v_ext[:, kb, h * 65:h * 65 + 65],
                    start=(i == 0), stop=(i == len(kbs) - 1),
                )
            rr = sm_pool.tile([P, 1], F32, tag="rr", name="rr")
            nc.vector.reciprocal(rr[:], o_ps[:, HD:HD + 1])
            oq = sm_pool.tile([P, HD], F32R, tag="oq", name="oq")
            nc.vector.tensor_scalar_mul(oq[:], o_ps[:, 0:HD], rr[:])
            oq_store[(qt, h)] = oq

        def phase_c2(qt, ot_store, oq_store):
            ot = ot_store.pop(qt)
            for h in range(NH):
                cc, po = h // 2, (h % 2) * HD
                oq = oq_store.pop((qt, h))
                # t region for head h: cols [384+128h, 512+128h) f32
                t_ps = ot[0:HD, 384 + P * h:384 + P * (h + 1)].bitcast(F32R)
                nc.tensor.transpose(t_ps, oq[:], ident[:])
                dst = oT[po:po + HD, cc, qt * P:(qt + 1) * P]
                nc.vector.tensor_copy(dst, t_ps)

        def phase_d(qt):
            y_sb = ysb_pool.tile([P, D], F32, tag="ysb", name="y_sb")
            for half in range(2):
                sl = slice(half * 512, (half + 1) * 512)
                y_ps = ppool.tile([P, 512], F32, tag="pqk", name="y_ps")
                for cc in range(2):
                    nc.tensor.matmul(
                        y_ps[:],
                        lhsT=oT[:, cc, qt * P:(qt + 1) * P],
                        rhs=wo_sb[:, cc, sl],
                        start=(cc == 0), stop=(cc == 1),
                    )
                if half == 0:
                    nc.vector.tensor_copy(y_sb[:, sl], y_ps[:])
                else:
                    nc.scalar.copy(y_sb[:, sl], y_ps[:])
            nc.sync.dma_start(y_d[qt * P:(qt + 1) * P, :], y_sb[:])

        # ---- pipelined emission: all heads advance together -------------
        # step kb: B(h,kb) x4 | C1(h,kb-4) x4 | C2(kb-4, all h) | D(kb-5)
        # projection waves and V blocks are woven into the steps.
        C1_LAG, D_LAG = 4, 5

        for fc in (0, 2, 1, 3):
            qk_proj(fc, 0)
        e_tiles, ot_store, oq_store = {}, {}, {}

        def inject_for(kb):
            items = []
            t = kb // 4 + 1
            if t <= 3:
                idx = kb % 4
                if idx == 0:
                    items.append(lambda: qk_proj(0, t))
                    items.append(lambda: qk_proj(2, t))
                elif idx == 1:
                    items.append(lambda: qk_proj(1, t))
                    items.append(lambda: qk_proj(3, t))
            if kb < KB - 2:
                items.append(lambda: v_proj(kb + 2))
            return items

        v_proj(0)
        v_proj(1)
        for kb in range(KB):
            inj = inject_for(kb)
            for h in range(NH):
                phase_b(h, kb, e_tiles)
                if h < len(inj):
                    inj[h]()
                if kb >= C1_LAG:
                    phase_c1(h, kb - C1_LAG, e_tiles, ot_store, oq_store)
            for f in inj[NH:]:
                f()
            if kb >= C1_LAG:
                phase_c2(kb - C1_LAG, ot_store, oq_store)
            if kb >= D_LAG:
                phase_d(kb - D_LAG)
        for qt in range(KB - C1_LAG, KB):
            for h in range(NH):
                phase_c1(h, qt, e_tiles, ot_store, oq_store)
            phase_c2(qt, ot_store, oq_store)
            phase_d(qt - 1)
        phase_d(KB - 1)


_NC_CACHE = None


def _get_nc():
    global _NC_CACHE
    if _NC_CACHE is None:
        _NC_CACHE = _build_nc()
    return _NC_CACHE


def kernel(x, qkv_w, qkv_b, out_w, out_b):
    x = np.asarray(x, dtype=np.float32)
    qkv_w = np.asarray(qkv_w, dtype=np.float32)
    qkv_b = np.asarray(qkv_b, dtype=np.float32)
    out_w = np.asarray(out_w, dtype=np.float32)
    out_b = np.asarray(out_b, dtype=np.float32)
    B = x.shape[0]
    assert x.shape == (B, L, D) and B * 4 == N_CORES

    nc = _get_nc()

    xts = [np.ascontiguousarray(x[b].T.astype(np.float16)) for b in range(B)]
    in_maps = []
    for core in range(N_CORES):
        b, g = divmod(core, 4)
        rq = slice(g * FV, (g + 1) * FV)
        rk = slice(D + g * FV, D + (g + 1) * FV)
        rv = slice(2 * D + g * FV, 2 * D + (g + 1) * FV)
        wqk_t = np.ascontiguousarray(
            np.concatenate([qkv_w[rq], qkv_w[rk]], axis=0).T)      # [D, 512]
        wqk_fc = np.ascontiguousarray(
            wqk_t.reshape(D, 4, P).transpose(1, 0, 2).astype(np.float16))
        wv_t = np.ascontiguousarray(qkv_w[rv].T.astype(np.float16))
        wo_t = np.ascontiguousarray(out_w[:, g * FV:(g + 1) * FV].T)
        bqk = np.ascontiguousarray(
            np.concatenate([qkv_b[rq], qkv_b[rk]]).reshape(4, P).T)
        bv = np.ascontiguousarray(qkv_b[rv].reshape(1, FV))
        in_maps.append({
            "xt": xts[b], "wqk_t": wqk_fc, "wv_t": wv_t, "wo_t": wo_t,
            "bqk": bqk, "bv": bv,
        })

    res = run_bass_kernel_spmd(nc, in_maps, list(range(N_CORES)))
    y = np.empty((B, L, D), dtype=np.float32)
    for b in range(B):
        acc = res.results[b * 4 + 0]["y"].astype(np.float32)
        for g in range(1, 4):
            acc = acc + res.results[b * 4 + g]["y"]
        y[b] = acc
    if np.any(out_b):
        y += out_b
    return y


# revision 19
# speedup vs baseline: 1.4381x; 1.4381x over previous
"""Sliding-window MHA Trainium2 kernel, sharded over 8 NeuronCores.

Problem (hardcoded): B=2, L=2048, D=1024, H=16 heads (hd=64), window
|i-j| <= 256, fp32 I/O.

Sharding: core = b*4 + g  (b in 0..1 batches, g in 0..3 head-groups of 4
heads). Each core computes QKV projection for its 4 heads, banded
attention, and a partial output projection (its 256 columns of the head
concat). Host sums the 4 partial projections per batch and adds out_b.

Device pipeline per core (matmuls fp32r, E/V fp16):
  x^T (host-pretransposed) -> Q^T,K^T [f,t] and V [t,f] projections
  per head: per k-block S^T = K^T.T Q^T over the 640-wide band window,
  E^T = exp(S/8) fp16 with static triangle masks on edge strips,
  O [q,65] = E^T-chunks.T @ [V|1] accumulated in PSUM, normalized by
  col 64, PE-transposed to O'^T, projected with the out_w slice.
  All phases are software-pipelined in emission order so PE (matmul),
  ACT (exp), DVE (copies) and DMA overlap.
"""

import numpy as np

import concourse.bacc as bacc
import concourse.mybir as mybir
import concourse.tile as tile
from concourse.bass_utils import run_bass_kernel_spmd
from concourse.masks import make_identity

F32 = mybir.dt.float32
F32R = mybir.dt.float32r
F16 = mybir.dt.float16

P = 128
L = 2048
D = 1024
NH = 4          # heads per core
HD = 64
FQK = 512       # q+k feature rows per core (4 heads * 64 * 2)
FV = 256        # v feature rows per core
WIN = 256
KB = L // P     # 16 k-blocks
N_CORES = 8


def _window(kb):
    """q-range [qlo, qhi) covered by k-block kb under |q-k| <= WIN."""
    k0 = kb * P
    qlo = max(0, k0 - WIN)
    qhi = min(L, k0 + P + WIN)
    return qlo, qhi - qlo


def _build_nc():
    nc = bacc.Bacc(
        "TRN2", target_bir_lowering=False, debug=False, num_devices=N_CORES
    )
    xt_d = nc.dram_tensor("xt", [D, L], F16, kind="ExternalInput").ap()
    wqk_d = nc.dram_tensor("wqk_t", [4, D, P], F16, kind="ExternalInput").ap()
    wv_d = nc.dram_tensor("wv_t", [D, FV], F16, kind="ExternalInput").ap()
    wo_d = nc.dram_tensor("wo_t", [FV, D], F32R, kind="ExternalInput").ap()
    bqk_d = nc.dram_tensor("bqk", [P, 4], F32, kind="ExternalInput").ap()
    bv_d = nc.dram_tensor("bv", [1, FV], F32, kind="ExternalInput").ap()
    y_d = nc.dram_tensor("y", [L, D], F32, kind="ExternalOutput").ap()

    with tile.TileContext(nc) as tc:
        _emit(nc, tc, xt_d, wqk_d, wv_d, wo_d, bqk_d, bv_d, y_d)
    nc.compile()
    return nc


def _emit(nc, tc, xt_d, wqk_d, wv_d, wo_d, bqk_d, bv_d, y_d):
    import contextlib

    ctx = contextlib.ExitStack()
    with ctx:
        const = ctx.enter_context(tc.tile_pool(name="const", bufs=1))
        w_pool = ctx.enter_context(tc.tile_pool(name="w", bufs=1))
        qk_pool = ctx.enter_context(tc.tile_pool(name="qk", bufs=1))
        v_pool = ctx.enter_context(tc.tile_pool(name="v", bufs=1))
        xt_pool = ctx.enter_context(tc.tile_pool(name="xt", bufs=1))
        e_pool = ctx.enter_context(tc.tile_pool(name="e", bufs=8))
        oT_pool = ctx.enter_context(tc.tile_pool(name="oT", bufs=1))
        sm_pool = ctx.enter_context(tc.tile_pool(name="sm", bufs=3))
        ysb_pool = ctx.enter_context(tc.tile_pool(name="ysb", bufs=3))
        ppool = ctx.enter_context(tc.tile_pool(name="ppsum", bufs=2, space="PSUM"))

        # ---- weight/bias/x DMAs (fine-grained, pipeline-ordered) --------
        wqk_sb = w_pool.tile([P, 4, 8, P], F16)
        wv_sb = w_pool.tile([P, 8, FV], F16)
        wo_sb = w_pool.tile([P, 2, D], F32R)
        xt_sb = xt_pool.tile([P, 8, L], F16)
        wqk_re = wqk_d.rearrange("f (c p) n -> p f c n", p=P)
        xt_re = xt_d.rearrange("(c p) t -> p c t", p=P)

        bqk_sb = const.tile([P, 4], F32)
        nc.sync.dma_start(bqk_sb[:], bqk_d[:])
        bv_row = const.tile([1, FV], F32)
        nc.sync.dma_start(bv_row[:], bv_d[:])

        # first compute wave needs wqk fc0/fc2 + xt t-slice 0
        nc.sync.dma_start(wqk_sb[:, 0, :, :], wqk_re[:, 0, :, :])
        for dc in range(8):
            nc.sync.dma_start(xt_sb[:, dc, 0:512], xt_re[:, dc, 0:512])
        nc.sync.dma_start(wqk_sb[:, 2, :, :], wqk_re[:, 2, :, :])
        nc.sync.dma_start(wv_sb[:], wv_d.rearrange("(c p) n -> p c n", p=P))
        for t in range(1, 4):
            for dc in range(8):
                nc.sync.dma_start(
                    xt_sb[:, dc, t * 512:(t + 1) * 512],
                    xt_re[:, dc, t * 512:(t + 1) * 512])
        for fc in (1, 3):
            nc.sync.dma_start(wqk_sb[:, fc, :, :], wqk_re[:, fc, :, :])
        nc.sync.dma_start(wo_sb[:], wo_d.rearrange("(c p) n -> p c n", p=P))

        # ---- constants --------------------------------------------------
        ident_f32 = const.tile([P, P], F32)
        make_identity(nc, ident_f32[:])
        ident = const.tile([P, P], F32R)
        nc.vector.tensor_copy(ident[:], ident_f32[:])
        # mask_l[kl, c] = 1 if c >= kl else 0   (upper tri incl diag)
        mask_l = const.tile([P, P], F16)
        nc.gpsimd.memset(mask_l[:], 1.0)
        nc.gpsimd.affine_select(
            out=mask_l[:], in_=mask_l[:],
            compare_op=mybir.AluOpType.is_ge, fill=0.0,
            base=0, pattern=[[1, P]], channel_multiplier=-1,
        )
        # mask_r[kl, c] = 1 if c <= kl else 0   (lower tri incl diag)
        mask_r = const.tile([P, P], F16)
        nc.gpsimd.memset(mask_r[:], 1.0)
        nc.gpsimd.affine_select(
            out=mask_r[:], in_=mask_r[:],
            compare_op=mybir.AluOpType.is_ge, fill=0.0,
            base=0, pattern=[[-1, P]], channel_multiplier=1,
        )
        bv_bc = const.tile([P, FV], F32)
        nc.gpsimd.partition_broadcast(bv_bc[:], bv_row[:])

        # Q^T / K^T per head-pair chunk: [f%128, chunk, t]
        qT = qk_pool.tile([P, 2, L], F32R)
        kT = qk_pool.tile([P, 2, L], F32R)
        # V per k-block, heads side by side, each with ones col (rowsum)
        v_ext = v_pool.tile([P, KB, NH * (HD + 1)], F16)
        nc.gpsimd.memset(v_ext[:], 1.0)
        oT = oT_pool.tile([P, 2, L], F32R)

        # ---- emission helpers -------------------------------------------
        def qk_proj(fc, t):
            pq = ppool.tile([P, 512], F32, tag="pqk", name="pq")
            for dc in range(8):
                nc.tensor.matmul(
                    pq[:],
                    lhsT=wqk_sb[:, fc, dc, :],
                    rhs=xt_sb[:, dc, t * 512:(t + 1) * 512],
                    start=(dc == 0), stop=(dc == 7),
                )
            dest = qT if fc < 2 else kT
            nc.vector.tensor_scalar_add(
                dest[:, fc % 2, t * 512:(t + 1) * 512], pq[:],
                bqk_sb[:, fc:fc + 1],
            )

        def v_proj(t):
            pv = ppool.tile([P, 512], F32, tag="pqk", name="pv")
            for dc in range(8):
                nc.tensor.matmul(
                    pv[:, 0:FV],
                    lhsT=xt_sb[:, dc, t * P:(t + 1) * P],
                    rhs=wv_sb[:, dc, :],
                    start=(dc == 0), stop=(dc == 7),
                )
            nc.vector.tensor_add(
                v_ext[:, t, :].rearrange("p (h c) -> p h c", h=NH)[:, :, 0:HD],
                pv[:, 0:FV].rearrange("p (h c) -> p h c", h=NH),
                bv_bc[:].rearrange("p (h c) -> p h c", h=NH),
            )

        def phase_b(h, kb, e_tiles):
            cc, po = h // 2, (h % 2) * HD
            qlo, w = _window(kb)
            e_sb = e_pool.tile([P, 640], F16, tag="e", name="e_sb")
            e_tiles[kb] = e_sb
            s_ps = ppool.tile([P, 1024], F32, tag="s", name="s_ps")
            if w == 640:
                pieces = [(0, 0, 320), (320, 512, 320)]
            else:
                pieces = [(0, 0, w)]
            for qoff, poff, pw in pieces:
                nc.tensor.matmul(
                    s_ps[:, poff:poff + pw],
                    lhsT=kT[po:po + HD, cc, kb * P:(kb + 1) * P],
                    rhs=qT[po:po + HD, cc, qlo + qoff:qlo + qoff + pw],
                    start=True, stop=True,
                )
            if w == 640:
                src = s_ps[:].rearrange("p (g c) -> p g c", g=2)[:, :, 0:320]
                dst = e_sb[:].rearrange("p (g c) -> p g c", g=2)
            else:
                src = s_ps[:, 0:w]
                dst = e_sb[0:P, 0:w]
            nc.scalar.activation(
                dst, src, mybir.ActivationFunctionType.Exp, scale=0.125)
            if kb >= 2:
                nc.gpsimd.tensor_mul(e_sb[:, 0:P], e_sb[:, 0:P], mask_l[:])
            if kb <= KB - 3:
                nc.gpsimd.tensor_mul(
                    e_sb[:, w - P:w], e_sb[:, w - P:w], mask_r[:])

        def phase_c1(h, qt, e_tiles, store):
            cc, po = h // 2, (h % 2) * HD
            kbs = range(max(0, qt - 2), min(KB, qt + 3))
            ot = ppool.tile([P, 512], F32, tag="ot", name="ot")
            o_ps = ot[:, 0:HD + 1]
            for i, kb in enumerate(kbs):
                qlo, w = _window(kb)
                off = qt * P - qlo
                nc.tensor.matmul(
                    o_ps,
                    lhsT=e_tiles[kb][:, off:off + P],
                    rhs=v_ext[:, kb, h * 65:h * 65 + 65],
                    start=(i == 0), stop=(i == len(kbs) - 1),
                )
            rr = sm_pool.tile([P, 1], F32, tag="rr", name="rr")
            nc.vector.reciprocal(rr[:], o_ps[:, HD:HD + 1])
            oq = sm_pool.tile([P, HD], F32R, tag="oq", name="oq")
            nc.vector.tensor_scalar_mul(oq[:], o_ps[:, 0:HD], rr[:])
            store[qt] = (ot, oq)

        def phase_c2(h, qt, store):
            cc, po = h // 2, (h % 2) * HD
            ot, oq = store.pop(qt)
            t_ps = ot[0:HD, 128:256].bitcast(F32R)
            nc.tensor.transpose(t_ps, oq[:], ident[:])
            dst = oT[po:po + HD, cc, qt * P:(qt + 1) * P]
            nc.vector.tensor_copy(dst, t_ps)

        def phase_d(qt):
            y_sb = ysb_pool.tile([P, D], F32, tag="ysb", name="y_sb")
            for half in range(2):
                sl = slice(half * 512, (half + 1) * 512)
                y_ps = ppool.tile([P, 512], F32, tag="pqk", name="y_ps")
                for cc in range(2):
                    nc.tensor.matmul(
                        y_ps[:],
                        lhsT=oT[:, cc, qt * P:(qt + 1) * P],
                        rhs=wo_sb[:, cc, sl],
                        start=(cc == 0), stop=(cc == 1),
                    )
                if half == 0:
                    nc.vector.tensor_copy(y_sb[:, sl], y_ps[:])
                else:
                    nc.scalar.copy(y_sb[:, sl], y_ps[:])
            nc.sync.dma_start(y_d[qt * P:(qt + 1) * P, :], y_sb[:])

        # ---- pipelined emission ----------------------------------------
        # per-head step loop: B(kb) leads; C1 lags 4 (exp+mask slack),
        # C2 lags 5, D lags 6 (h3 only). Projections stream in as waves.
        C1_LAG, C2_LAG, D_LAG = 4, 5, 6

        def run_head(h, inject=None):
            e_tiles, store = {}, {}
            for kb in range(KB):
                phase_b(h, kb, e_tiles)
                if inject:
                    for f in inject.get(kb, ()):
                        f()
                if kb >= C1_LAG:
                    phase_c1(h, kb - C1_LAG, e_tiles, store)
                if kb >= C2_LAG:
                    phase_c2(h, kb - C2_LAG, store)
                if h == NH - 1 and kb >= D_LAG:
                    phase_d(kb - D_LAG)
            for qt in range(KB - C1_LAG, KB):
                phase_c1(h, qt, e_tiles, store)
                phase_c2(h, qt - 1, store)
                if h == NH - 1:
                    phase_d(qt - 2)
            phase_c2(h, KB - 1, store)
            if h == NH - 1:
                phase_d(KB - 2)
                phase_d(KB - 1)

        # h0: interleave qk (fc0/fc2) waves + all V projections.
        # B(h0,kb) needs qT cols up to kb*128+384 -> t-wave (kb+2)//4.
        inj0 = {}
        emitted_t = [0]
        qk_proj(0, 0)
        qk_proj(2, 0)
        v_proj(0)
        v_proj(1)
        for kb in range(KB):
            items = []
            t_need = min(3, (kb + 3 + 2) // 4)   # one wave ahead of need
            while emitted_t[0] < t_need:
                emitted_t[0] += 1
                tt = emitted_t[0]
                items.append(lambda tt=tt: qk_proj(0, tt))
                items.append(lambda tt=tt: qk_proj(2, tt))
            if kb + 2 < KB:
                items.append(lambda kb=kb: v_proj(kb + 2))
            inj0[kb] = items
        run_head(0, inj0)

        # h1: stream fc1/fc3 projections (needed by h2/h3), one per 2 steps
        inj1 = {}
        seq = [(1, 0), (3, 0), (1, 1), (3, 1), (1, 2), (3, 2), (1, 3), (3, 3)]
        for i, (fc, t) in enumerate(seq):
            inj1.setdefault(2 * i, []).append(lambda fc=fc, t=t: qk_proj(fc, t))
        run_head(1, inj1)
        run_head(2)
        run_head(3)


_NC_CACHE = None


def _get_nc():
    global _NC_CACHE
    if _NC_CACHE is None:
        _NC_CACHE = _build_nc()
    return _NC_CACHE


def kernel(x, qkv_w, qkv_b, out_w, out_b):
    x = np.asarray(x, dtype=np.float32)
    qkv_w = np.asarray(qkv_w, dtype=np.float32)
    qkv_b = np.asarray(qkv_b, dtype=np.float32)
    out_w = np.asarray(out_w, dtype=np.float32)
    out_b = np.asarray(out_b, dtype=np.float32)
    B = x.shape[0]
    assert x.shape == (B, L, D) and B * 4 == N_CORES

    nc = _get_nc()

    xts = [np.ascontiguousarray(x[b].T.astype(np.float16)) for b in range(B)]
    in_maps = []
    for core in range(N_CORES):
        b, g = divmod(core, 4)
        rq = slice(g * FV, (g + 1) * FV)
        rk = slice(D + g * FV, D + (g + 1) * FV)
        rv = slice(2 * D + g * FV, 2 * D + (g + 1) * FV)
        wqk_t = np.ascontiguousarray(
            np.concatenate([qkv_w[rq], qkv_w[rk]], axis=0).T)      # [D, 512]
        wqk_fc = np.ascontiguousarray(
            wqk_t.reshape(D, 4, P).transpose(1, 0, 2).astype(np.float16))
        wv_t = np.ascontiguousarray(qkv_w[rv].T.astype(np.float16))
        wo_t = np.ascontiguousarray(out_w[:, g * FV:(g + 1) * FV].T)
        bqk = np.ascontiguousarray(
            np.concatenate([qkv_b[rq], qkv_b[rk]]).reshape(4, P).T)
        bv = np.ascontiguousarray(qkv_b[rv].reshape(1, FV))
        in_maps.append({
            "xt": xts[b], "wqk_t": wqk_fc, "wv_t": wv_t, "wo_t": wo_t,
            "bqk": bqk, "bv": bv,
        })

    res = run_bass_kernel_spmd(nc, in_maps, list(range(N_CORES)))
    y = np.empty((B, L, D), dtype=np.float32)
    for b in range(B):
        acc = res.results[b * 4 + 0]["y"].astype(np.float32)
        for g in range(1, 4):
            acc = acc + res.results[b * 4 + g]["y"]
        y[b] = acc
    if np.any(out_b):
        y += out_b
    return y


# revision 20
# speedup vs baseline: 1.4619x; 1.0166x over previous
"""Sliding-window MHA Trainium2 kernel, sharded over 8 NeuronCores.

Problem (hardcoded): B=2, L=2048, D=1024, H=16 heads (hd=64), window
|i-j| <= 256, fp32 I/O.

Sharding: core = b*4 + g  (b in 0..1 batches, g in 0..3 head-groups of 4
heads). Each core computes QKV projection for its 4 heads, banded
attention, and a partial output projection (its 256 columns of the head
concat). Host sums the 4 partial projections per batch and adds out_b.

Device pipeline per core (matmuls fp32r, E/V fp16):
  x^T (host-pretransposed) -> Q^T,K^T [f,t] and V [t,f] projections
  per head: per k-block S^T = K^T.T Q^T over the 640-wide band window,
  E^T = exp(S/8) fp16 with static triangle masks on edge strips,
  O [q,65] = E^T-chunks.T @ [V|1] accumulated in PSUM, normalized by
  col 64, PE-transposed to O'^T, projected with the out_w slice.
  All phases are software-pipelined in emission order so PE (matmul),
  ACT (exp), DVE (copies) and DMA overlap.
"""

import numpy as np

import concourse.bacc as bacc
import concourse.mybir as mybir
import concourse.tile as tile
from concourse.bass_utils import run_bass_kernel_spmd
from concourse.masks import make_identity

F32 = mybir.dt.float32
F32R = mybir.dt.float32r
F16 = mybir.dt.float16

P = 128
L = 2048
D = 1024
NH = 4          # heads per core
HD = 64
FQK = 512       # q+k feature rows per core (4 heads * 64 * 2)
FV = 256        # v feature rows per core
WIN = 256
KB = L // P     # 16 k-blocks
N_CORES = 8


def _window(kb):
    """q-range [qlo, qhi) covered by k-block kb under |q-k| <= WIN."""
    k0 = kb * P
    qlo = max(0, k0 - WIN)
    qhi = min(L, k0 + P + WIN)
    return qlo, qhi - qlo


def _build_nc():
    nc = bacc.Bacc(
        "TRN2", target_bir_lowering=False, debug=False, num_devices=N_CORES
    )
    xt_d = nc.dram_tensor("xt", [D, L], F16, kind="ExternalInput").ap()
    wqk_d = nc.dram_tensor("wqk_t", [4, D, P], F16, kind="ExternalInput").ap()
    wv_d = nc.dram_tensor("wv_t", [D, FV], F16, kind="ExternalInput").ap()
    wo_d = nc.dram_tensor("wo_t", [FV, D], F32R, kind="ExternalInput").ap()
    bqk_d = nc.dram_tensor("bqk", [P, 4], F32, kind="ExternalInput").ap()
    bv_d = nc.dram_tensor("bv", [1, FV], F32, kind="ExternalInput").ap()
    y_d = nc.dram_tensor("y", [L, D], F32, kind="ExternalOutput").ap()

    with tile.TileContext(nc) as tc:
        _emit(nc, tc, xt_d, wqk_d, wv_d, wo_d, bqk_d, bv_d, y_d)
    nc.compile()
    return nc


def _emit(nc, tc, xt_d, wqk_d, wv_d, wo_d, bqk_d, bv_d, y_d):
    import contextlib

    ctx = contextlib.ExitStack()
    with ctx:
        const = ctx.enter_context(tc.tile_pool(name="const", bufs=1))
        w_pool = ctx.enter_context(tc.tile_pool(name="w", bufs=1))
        qk_pool = ctx.enter_context(tc.tile_pool(name="qk", bufs=1))
        v_pool = ctx.enter_context(tc.tile_pool(name="v", bufs=1))
        xt_pool = ctx.enter_context(tc.tile_pool(name="xt", bufs=1))
        e_pool = ctx.enter_context(tc.tile_pool(name="e", bufs=8))
        oT_pool = ctx.enter_context(tc.tile_pool(name="oT", bufs=1))
        sm_pool = ctx.enter_context(tc.tile_pool(name="sm", bufs=3))
        ysb_pool = ctx.enter_context(tc.tile_pool(name="ysb", bufs=3))
        opr_pool = ctx.enter_context(tc.tile_pool(name="opr", bufs=20))
        ppool = ctx.enter_context(tc.tile_pool(name="ppsum", bufs=2, space="PSUM"))

        # ---- weight/bias/x DMAs (fine-grained, pipeline-ordered) --------
        wqk_sb = w_pool.tile([P, 4, 8, P], F16)
        wv_sb = w_pool.tile([P, 8, FV], F16)
        wo_sb = w_pool.tile([P, 2, D], F32R)
        xt_sb = xt_pool.tile([P, 8, L], F16)
        wqk_re = wqk_d.rearrange("f (c p) n -> p f c n", p=P)
        xt_re = xt_d.rearrange("(c p) t -> p c t", p=P)

        bqk_sb = const.tile([P, 4], F32)
        nc.sync.dma_start(bqk_sb[:], bqk_d[:])
        bv_row = const.tile([1, FV], F32)
        nc.sync.dma_start(bv_row[:], bv_d[:])

        # first compute wave needs wqk fc0/fc2 + xt t-slice 0
        nc.sync.dma_start(wqk_sb[:, 0, :, :], wqk_re[:, 0, :, :])
        for dc in range(8):
            nc.sync.dma_start(xt_sb[:, dc, 0:512], xt_re[:, dc, 0:512])
        nc.sync.dma_start(wqk_sb[:, 2, :, :], wqk_re[:, 2, :, :])
        nc.sync.dma_start(wv_sb[:], wv_d.rearrange("(c p) n -> p c n", p=P))
        for t in range(1, 4):
            for dc in range(8):
                nc.sync.dma_start(
                    xt_sb[:, dc, t * 512:(t + 1) * 512],
                    xt_re[:, dc, t * 512:(t + 1) * 512])
        for fc in (1, 3):
            nc.sync.dma_start(wqk_sb[:, fc, :, :], wqk_re[:, fc, :, :])
        nc.sync.dma_start(wo_sb[:], wo_d.rearrange("(c p) n -> p c n", p=P))

        # ---- constants --------------------------------------------------
        ident_f32 = const.tile([P, P], F32)
        make_identity(nc, ident_f32[:])
        ident = const.tile([P, P], F32R)
        nc.vector.tensor_copy(ident[:], ident_f32[:])
        # mask_l[kl, c] = 1 if c >= kl else 0   (upper tri incl diag)
        mask_l = const.tile([P, P], F16)
        nc.gpsimd.memset(mask_l[:], 1.0)
        nc.gpsimd.affine_select(
            out=mask_l[:], in_=mask_l[:],
            compare_op=mybir.AluOpType.is_ge, fill=0.0,
            base=0, pattern=[[1, P]], channel_multiplier=-1,
        )
        # mask_r[kl, c] = 1 if c <= kl else 0   (lower tri incl diag)
        mask_r = const.tile([P, P], F16)
        nc.gpsimd.memset(mask_r[:], 1.0)
        nc.gpsimd.affine_select(
            out=mask_r[:], in_=mask_r[:],
            compare_op=mybir.AluOpType.is_ge, fill=0.0,
            base=0, pattern=[[-1, P]], channel_multiplier=1,
        )
        bv_bc = const.tile([P, FV], F32)
        nc.gpsimd.partition_broadcast(bv_bc[:], bv_row[:])

        # Q^T / K^T per head-pair chunk: [f%128, chunk, t]
        qT = qk_pool.tile([P, 2, L], F32R)
        kT = qk_pool.tile([P, 2, L], F32R)
        # V per k-block, heads side by side, each with ones col (rowsum)
        v_ext = v_pool.tile([P, KB, NH * (HD + 1)], F16)
        nc.gpsimd.memset(v_ext[:], 1.0)
        oT = oT_pool.tile([P, 2, L], F32R)
        pair_store = {}

        # ---- emission helpers -------------------------------------------
        def qk_proj(fc, t):
            pq = ppool.tile([P, 512], F32, tag="pqk", name="pq")
            for dc in range(8):
                nc.tensor.matmul(
                    pq[:],
                    lhsT=wqk_sb[:, fc, dc, :],
                    rhs=xt_sb[:, dc, t * 512:(t + 1) * 512],
                    start=(dc == 0), stop=(dc == 7),
                )
            dest = qT if fc < 2 else kT
            nc.vector.tensor_scalar_add(
                dest[:, fc % 2, t * 512:(t + 1) * 512], pq[:],
                bqk_sb[:, fc:fc + 1],
            )

        def v_proj(t):
            pv = ppool.tile([P, 512], F32, tag="pqk", name="pv")
            for dc in range(8):
                nc.tensor.matmul(
                    pv[:, 0:FV],
                    lhsT=xt_sb[:, dc, t * P:(t + 1) * P],
                    rhs=wv_sb[:, dc, :],
                    start=(dc == 0), stop=(dc == 7),
                )
            nc.vector.tensor_add(
                v_ext[:, t, :].rearrange("p (h c) -> p h c", h=NH)[:, :, 0:HD],
                pv[:, 0:FV].rearrange("p (h c) -> p h c", h=NH),
                bv_bc[:].rearrange("p (h c) -> p h c", h=NH),
            )

        def phase_b(h, kb, e_tiles):
            cc, po = h // 2, (h % 2) * HD
            qlo, w = _window(kb)
            e_sb = e_pool.tile([P, 640], F16, tag="e", name="e_sb")
            e_tiles[kb] = e_sb
            s_ps = ppool.tile([P, 1024], F32, tag="s", name="s_ps")
            if w == 640:
                pieces = [(0, 0, 320), (320, 512, 320)]
            else:
                pieces = [(0, 0, w)]
            for qoff, poff, pw in pieces:
                nc.tensor.matmul(
                    s_ps[:, poff:poff + pw],
                    lhsT=kT[po:po + HD, cc, kb * P:(kb + 1) * P],
                    rhs=qT[po:po + HD, cc, qlo + qoff:qlo + qoff + pw],
                    start=True, stop=True,
                )
            if w == 640:
                src = s_ps[:].rearrange("p (g c) -> p g c", g=2)[:, :, 0:320]
                dst = e_sb[:].rearrange("p (g c) -> p g c", g=2)
            else:
                src = s_ps[:, 0:w]
                dst = e_sb[0:P, 0:w]
            nc.scalar.activation(
                dst, src, mybir.ActivationFunctionType.Exp, scale=0.125)
            if kb >= 2:
                nc.gpsimd.tensor_mul(e_sb[:, 0:P], e_sb[:, 0:P], mask_l[:])
            if kb <= KB - 3:
                nc.gpsimd.tensor_mul(
                    e_sb[:, w - P:w], e_sb[:, w - P:w], mask_r[:])

        def phase_c1(h, qt, e_tiles, store):
            cc, po = h // 2, (h % 2) * HD
            kbs = range(max(0, qt - 2), min(KB, qt + 3))
            ot = ppool.tile([P, 512], F32, tag="ot", name="ot")
            o_ps = ot[:, 0:HD + 1]
            for i, kb in enumerate(kbs):
                qlo, w = _window(kb)
                off = qt * P - qlo
                nc.tensor.matmul(
                    o_ps,
                    lhsT=e_tiles[kb][:, off:off + P],
                    rhs=v_ext[:, kb, h * 65:h * 65 + 65],
                    start=(i == 0), stop=(i == len(kbs) - 1),
                )
            rr = sm_pool.tile([P, 1], F32, tag="rr", name="rr")
            nc.vector.reciprocal(rr[:], o_ps[:, HD:HD + 1])
            # normalized head-output into its half of the pair tile [128,128]
            if h % 2 == 0:
                opr = opr_pool.tile([P, P], F32R, tag="opr", name="opr")
                pair_store[qt] = opr
            else:
                opr = pair_store[qt]
            nc.vector.tensor_scalar_mul(
                opr[:, po:po + HD], o_ps[:, 0:HD], rr[:])
            store[qt] = ot

        def phase_c2(h, qt, store):
            # h odd: transpose the completed [128,128] head-pair tile
            cc = h // 2
            ot = store.pop(qt)
            opr = pair_store.pop(qt)
            t_ps = ot[:, 128:256].bitcast(F32R)
            nc.tensor.transpose(t_ps, opr[:], ident[:])
            nc.vector.tensor_copy(oT[:, cc, qt * P:(qt + 1) * P], t_ps)

        def phase_d(qt):
            y_sb = ysb_pool.tile([P, D], F32, tag="ysb", name="y_sb")
            for half in range(2):
                sl = slice(half * 512, (half + 1) * 512)
                y_ps = ppool.tile([P, 512], F32, tag="pqk", name="y_ps")
                for cc in range(2):
                    nc.tensor.matmul(
                        y_ps[:],
                        lhsT=oT[:, cc, qt * P:(qt + 1) * P],
                        rhs=wo_sb[:, cc, sl],
                        start=(cc == 0), stop=(cc == 1),
                    )
                if half == 0:
                    nc.vector.tensor_copy(y_sb[:, sl], y_ps[:])
                else:
                    nc.scalar.copy(y_sb[:, sl], y_ps[:])
            nc.sync.dma_start(y_d[qt * P:(qt + 1) * P, :], y_sb[:])

        # ---- pipelined emission ----------------------------------------
        # per-head step loop: B(kb) leads; C1 lags 4 (exp+mask slack),
        # C2 lags 5, D lags 6 (h3 only). Projections stream in as waves.
        C1_LAG, C2_LAG, D_LAG = 4, 5, 6

        def run_head(h, inject=None):
            e_tiles, store = {}, {}
            for kb in range(KB):
                phase_b(h, kb, e_tiles)
                if inject:
                    for f in inject.get(kb, ()):
                        f()
                if kb >= C1_LAG:
                    phase_c1(h, kb - C1_LAG, e_tiles, store)
                if h % 2 == 1 and kb >= C2_LAG:
                    phase_c2(h, kb - C2_LAG, store)
                if h == NH - 1 and kb >= D_LAG:
                    phase_d(kb - D_LAG)
            for qt in range(KB - C1_LAG, KB):
                phase_c1(h, qt, e_tiles, store)
                if h % 2 == 1:
                    phase_c2(h, qt - 1, store)
                    if h == NH - 1:
                        phase_d(qt - 2)
            if h % 2 == 1:
                phase_c2(h, KB - 1, store)
            if h == NH - 1:
                phase_d(KB - 2)
                phase_d(KB - 1)

        # h0: interleave qk (fc0/fc2) waves + all V projections.
        # B(h0,kb) needs qT cols up to kb*128+384 -> t-wave (kb+2)//4.
        inj0 = {}
        emitted_t = [0]
        qk_proj(0, 0)
        qk_proj(2, 0)
        v_proj(0)
        v_proj(1)
        for kb in range(KB):
            items = []
            t_need = min(3, (kb + 3 + 2) // 4)   # one wave ahead of need
            while emitted_t[0] < t_need:
                emitted_t[0] += 1
                tt = emitted_t[0]
                items.append(lambda tt=tt: qk_proj(0, tt))
                items.append(lambda tt=tt: qk_proj(2, tt))
            if kb + 2 < KB:
                items.append(lambda kb=kb: v_proj(kb + 2))
            inj0[kb] = items
        run_head(0, inj0)

        # h1: stream fc1/fc3 projections (needed by h2/h3), one per 2 steps
        inj1 = {}
        seq = [(1, 0), (3, 0), (1, 1), (3, 1), (1, 2), (3, 2), (1, 3), (3, 3)]
        for i, (fc, t) in enumerate(seq):
            inj1.setdefault(2 * i, []).append(lambda fc=fc, t=t: qk_proj(fc, t))
        run_head(1, inj1)
        run_head(2)
        run_head(3)


_NC_CACHE = None


def _get_nc():
    global _NC_CACHE
    if _NC_CACHE is None:
        _NC_CACHE = _build_nc()
    return _NC_CACHE


def kernel(x, qkv_w, qkv_b, out_w, out_b):
    x = np.asarray(x, dtype=np.float32)
    qkv_w = np.asarray(qkv_w, dtype=np.float32)
    qkv_b = np.asarray(qkv_b, dtype=np.float32)
    out_w = np.asarray(out_w, dtype=np.float32)
    out_b = np.asarray(out_b, dtype=np.float32)
    B = x.shape[0]
    assert x.shape == (B, L, D) and B * 4 == N_CORES

    nc = _get_nc()

    xts = [np.ascontiguousarray(x[b].T.astype(np.float16)) for b in range(B)]
    in_maps = []
    for core in range(N_CORES):
        b, g = divmod(core, 4)
        rq = slice(g * FV, (g + 1) * FV)
        rk = slice(D + g * FV, D + (g + 1) * FV)
        rv = slice(2 * D + g * FV, 2 * D + (g + 1) * FV)
        wqk_t = np.ascontiguousarray(
            np.concatenate([qkv_w[rq], qkv_w[rk]], axis=0).T)      # [D, 512]
        wqk_fc = np.ascontiguousarray(
            wqk_t.reshape(D, 4, P).transpose(1, 0, 2).astype(np.float16))
        wv_t = np.ascontiguousarray(qkv_w[rv].T.astype(np.float16))
        wo_t = np.ascontiguousarray(out_w[:, g * FV:(g + 1) * FV].T)
        bqk = np.ascontiguousarray(
            np.concatenate([qkv_b[rq], qkv_b[rk]]).reshape(4, P).T)
        bv = np.ascontiguousarray(qkv_b[rv].reshape(1, FV))
        in_maps.append({
            "xt": xts[b], "wqk_t": wqk_fc, "wv_t": wv_t, "wo_t": wo_t,
            "bqk": bqk, "bv": bv,
        })

    res = run_bass_kernel_spmd(nc, in_maps, list(range(N_CORES)))
    y = np.empty((B, L, D), dtype=np.float32)
    for b in range(B):
        acc = res.results[b * 4 + 0]["y"].astype(np.float32)
        for g in range(1, 4):
            acc = acc + res.results[b * 4 + g]["y"]
        y[b] = acc
    if np.any(out_b):
        y += out_b
    return y
